# revision 29
# baseline (speedup 1.0000x reference)
"""Trainium2 kernel for nn_Capture_Data: cap = sum(spec_data*filter, axis=(1,2))
plus Poisson/Gaussian noise synthesis.

Strategy:
  - The heavy, memory-bound part (reading 2 x 235MB and reducing over the 112
    (channel, spectral) slices) runs on 8 NeuronCores, one batch element per
    core (pure data parallel).
  - The accumulation is done with sequential fp32 adds in slice order 0..111,
    which reproduces XLA:CPU's column-reduction order bit-exactly.
  - The tiny noise-synthesis tail (jax.random.poisson/normal on the
    [8,256,256,1] result) is replicated with the exact same jax ops on the
    host CPU backend with threefry keys, matching the reference bit-for-bit.
"""

import numpy as np

P = 128             # SBUF partitions
CS = 112            # 4*28 reduced slices per batch element
NPIX = 256 * 256    # pixels per batch element
FREE = NPIX // P    # 512
G = 8               # slices loaded/multiplied per group
GROUPS = [G] * (CS // G)
N_CORES = 8

NL_IN = 0.5
CONS = 1e-10
POISSON_GAIN = 20.0

_TRACE = False      # set by test harness to collect an NTFF profile
LAST_EXEC_NS = None

_cached = {}


def _build_bass(groups=None, bufs_sf=4, bufs_p=2, pin_singles=True, bufs_1=8,
                part_major=True, init_by_mult=True, pool_mult_idx=()):
    import concourse.bacc as bacc
    import concourse.mybir as mybir
    from concourse.tile import TileContext
    from concourse.tile_rust import add_dep_helper

    nc = bacc.Bacc(None, target_bir_lowering=False)
    f32 = mybir.dt.float32
    if part_major:
        # host pre-permutes inputs to [P, CS*FREE] (partition-major), so each
        # group load is a long contiguous run per partition -> fat DMA
        # descriptors instead of 2KB ones
        spec = nc.dram_tensor("spec", [P, CS * FREE], f32, kind="ExternalInput")
        filt = nc.dram_tensor("filt", [P, CS * FREE], f32, kind="ExternalInput")
    else:
        spec = nc.dram_tensor("spec", [CS, NPIX], f32, kind="ExternalInput")
        filt = nc.dram_tensor("filt", [CS, NPIX], f32, kind="ExternalInput")
    cap = nc.dram_tensor("cap", [P, FREE], f32, kind="ExternalOutput")

    if groups is None:
        groups = list(GROUPS)
    assert sum(groups) == CS

    with TileContext(nc) as tc:
        with (
            tc.tile_pool(name="io_s", bufs=bufs_sf) as s_pool,
            tc.tile_pool(name="io_f", bufs=bufs_sf) as f_pool,
            tc.tile_pool(name="io_p", bufs=bufs_p) as p_pool,
            tc.tile_pool(name="io_1", bufs=bufs_1) as one_pool,
            tc.tile_pool(name="accp", bufs=1) as acc_pool,
        ):
            acc = acc_pool.tile([P, FREE], f32)
            if not init_by_mult:
                nc.vector.memset(acc[:], 0.0)
            cs0 = 0
            last_big_s = last_big_f = None
            for gi, gs in enumerate(groups):
                w = gs * FREE
                if gs == 1:
                    # single-slice groups: dedicated small tiles so neither
                    # the mults nor the add chain gate their loads
                    st = one_pool.tile([P, w], f32, tag="s1")
                    ft = one_pool.tile([P, w], f32, tag="f1")
                    prod = one_pool.tile([P, w], f32, tag="p1")
                else:
                    st = s_pool.tile([P, w], f32, tag="spec")
                    ft = f_pool.tile([P, w], f32, tag="filt")
                    prod = p_pool.tile([P, w], f32, tag="prod")
                # slice k of this group lands at free-dim columns
                # [k*FREE, (k+1)*FREE) in the canonical pixel layout:
                # pixel = partition*FREE + i
                if part_major:
                    src_s = spec[:, cs0 * FREE:(cs0 + gs) * FREE]
                    src_f = filt[:, cs0 * FREE:(cs0 + gs) * FREE]
                    dst_s, dst_f = st[:], ft[:]
                else:
                    src_s = spec[cs0:cs0 + gs, :].rearrange("k (p i) -> p k i", p=P)
                    src_f = filt[cs0:cs0 + gs, :].rearrange("k (p i) -> p k i", p=P)
                    dst_s = st[:].rearrange("p (k i) -> p k i", k=gs)
                    dst_f = ft[:].rearrange("p (k i) -> p k i", k=gs)
                # two HWDGE rings (SP + ACT) so the two streams transfer in parallel
                dma_s = nc.sync.dma_start(out=dst_s, in_=src_s)
                dma_f = nc.scalar.dma_start(out=dst_f, in_=src_f)
                if gs > 1:
                    last_big_s, last_big_f = dma_s, dma_f
                elif pin_singles and gi > 0 and last_big_s is not None:
                    # keep trailing single-slice loads from being hoisted
                    # early by the scheduler: order them after the last big
                    # group's loads so they arrive at the end of the stream
                    add_dep_helper(dma_s.ins, last_big_s.ins, sync=False,
                                   reason="tail single after big stream")
                    add_dep_helper(dma_f.ins, last_big_f.ins, sync=False,
                                   reason="tail single after big stream")
                mul_eng = nc.gpsimd if gi in pool_mult_idx else nc.vector
                k0 = 0
                if init_by_mult and gi == 0:
                    # start the chain by writing slice 0's product straight
                    # into acc (0.0 + t0 == t0 bitwise for these inputs)
                    nc.vector.tensor_mul(acc[:], st[:, 0:FREE], ft[:, 0:FREE])
                    if gs > 1:
                        mul_eng.tensor_mul(prod[:, FREE:], st[:, FREE:], ft[:, FREE:])
                    k0 = 1
                else:
                    mul_eng.tensor_mul(prod[:], st[:], ft[:])
                # sequential accumulation in global slice order => matches
                # XLA:CPU reduction order bit-exactly
                for k in range(k0, gs):
                    nc.vector.tensor_add(acc[:], acc[:], prod[:, k * FREE:(k + 1) * FREE])
                cs0 += gs

            nc.sync.dma_start(out=cap[:], in_=acc[:])
    nc.compile()
    return nc


def _ensure_trace_hook_importable():
    """bass_utils imports antenv.axon_hooks when tracing is requested (e.g.
    BASS_TRACE=1 in the environment). Some images ship antenv without that
    submodule; provide a functional shim so a trace request degrades
    gracefully instead of crashing."""
    try:
        import antenv.axon_hooks  # noqa: F401
        return
    except ImportError:
        pass
    try:
        import sys
        import types

        mod = types.ModuleType("antenv.axon_hooks")
        mod._hook = None
        mod.set_axon_ntff_profile_hook = lambda h: setattr(mod, "_hook", h)
        # returning None makes bass_utils skip tracing gracefully
        mod.get_axon_ntff_profile_hook = lambda: mod._hook
        sys.modules["antenv.axon_hooks"] = mod
        import antenv
        antenv.axon_hooks = mod
    except Exception:
        pass


def _run_device(spec_data, filt_data):
    """Run the Bass kernel on 8 cores; returns cap as float32 [8,256,256]."""
    global LAST_EXEC_NS
    from concourse.bass_utils import run_bass_kernel_spmd

    _ensure_trace_hook_importable()

    if "nc" not in _cached:
        _cached["nc"] = _build_bass()
    nc = _cached["nc"]

    in_maps = []
    for b in range(N_CORES):
        # permute to partition-major [P, CS*FREE] so device loads are long
        # contiguous runs per partition (fat DMA descriptors)
        sb = spec_data[b].reshape(CS, P, FREE).transpose(1, 0, 2)
        fb = filt_data[b].reshape(CS, P, FREE).transpose(1, 0, 2)
        in_maps.append({
            "spec": np.ascontiguousarray(sb).reshape(P, CS * FREE),
            "filt": np.ascontiguousarray(fb).reshape(P, CS * FREE),
        })
    res = run_bass_kernel_spmd(nc, in_maps, list(range(N_CORES)), trace=_TRACE)
    LAST_EXEC_NS = res.exec_time_ns
    out = np.empty((N_CORES, 256, 256), dtype=np.float32)
    for b in range(N_CORES):
        out[b] = np.asarray(res.results[b]["cap"]).reshape(256, 256)
    return out


def _noise_synthesis(cap_np):
    """Replicates the reference's jax ops bit-exactly on the CPU backend."""
    import jax
    import jax.numpy as jnp

    cpu = jax.devices("cpu")[0]
    with jax.default_device(cpu):
        cap = jnp.asarray(cap_np)  # [8,256,256,1] float32
        poisson_t = jnp.full_like(cap, POISSON_GAIN * NL_IN)
        dark_t = jnp.full_like(cap, 1.0 * NL_IN)
        gauss_t = jnp.full_like(cap, 1.0 * NL_IN)
        peak = cap + CONS

        key = jax.random.key(1, impl="threefry2x32")
        kp, kd, kg = jax.random.split(key, 3)
        pnoisy = jax.random.poisson(kp, peak).astype(cap.dtype)
        dnoisy = jax.random.poisson(kd, dark_t).astype(cap.dtype)
        gnoisy = jax.random.normal(kg, cap.shape, dtype=cap.dtype) * gauss_t

        noisy = (pnoisy + dnoisy + gnoisy) * poisson_t / 255.0
        return (
            np.asarray(noisy),
            np.asarray(peak),
            np.asarray(dark_t),
            np.asarray(gauss_t ** 2),
        )


def kernel(spec_data, filter):
    spec_data = np.asarray(spec_data, dtype=np.float32)
    filt = np.asarray(filter, dtype=np.float32)
    cap = _run_device(spec_data, filt)[..., None]  # [8,256,256,1]
    return _noise_synthesis(cap)
